# revision 7
# baseline (speedup 1.0000x reference)
"""Trainium2 Bass kernel for nn_MultiHeadAttention_47579647705431.

Multi-head attention (8 heads, dim 512, seq 1024, batch 16) with:
  - shared key/query linear (key_query_same=True: q and k both use Wk/bk)
  - causal (or arbitrary block-structured) mask
  - SimpleKT zero_pad: attention row 0 zeroed => out[:, 0, :] = bo

Sharding: data-parallel over batch across 8 NeuronCores (2 batches/core).
Per-core device pipeline (all matmuls bf16, fp32 PSUM accumulation):
  host: cast+transpose q/k/v slices and weights to bf16 feature-major
  1. kp/qp = Wk.T-stationary projections -> feature-major [o, n] bf16
  2. vp   = token-major projection [n, o] bf16 with interleaved ones
           columns (stride-65 layout) for the denominator ones-row trick
  3. per (batch, head-pair, s-chunk): scores^T [t, s] via K=64 row-packed
     matmuls, additive -1e9 mask via identity-matmul accumulation on mixed
     blocks, exp on ACT (scale=1/8 folded), AV via [t, 65]-stationary
     matmuls accumulating [65, 512] PSUM whose row 64 is the softmax
     denominator; reciprocal + K=1 ones broadcast matmuls + DVE multiply
     normalize into concat^T bf16
  4. final projection (concat^T-stationary) -> [n, o] fp32 -> DRAM

The walrus build here supports ONE sync wait per instruction; Tile emits
more. legalize_waits() hoists extra waits onto same-engine NoOps.
"""

import os
from contextlib import ExitStack

import numpy as np
import ml_dtypes

import concourse.bass as bass
import concourse.mybir as mybir
import concourse.tile as tile
from concourse.bass_utils import run_bass_kernel_spmd

F32 = mybir.dt.float32
BF16 = mybir.dt.bfloat16
BF = ml_dtypes.bfloat16

B, S, D, H, DH = 16, 1024, 512, 8, 64
NCORES = 8
BL = B // NCORES          # batches per core
N = BL * S                # tokens per core
NB = S // 128             # 128-blocks per sequence (8)
HP = H // 2               # head pairs (= o-blocks of 128)
NEG = -1.0e9

LAST_SIM_NS = None
LAST_EXEC_NS = None


def legalize_waits(nc):
    """Split multi-wait instructions: keep one wait, hoist the rest onto
    preceding same-engine NoOps (this walrus encodes 1 wait/instruction)."""
    for f in nc.m.functions:
        for blk in f.blocks:
            il = blk.instructions
            i = 0
            while i < len(il):
                inst = il[i]
                si = inst.sync_info
                if si is not None and si.on_wait and len(si.on_wait) > 1:
                    waits = list(si.on_wait)
                    for j, w in enumerate(waits[:-1]):
                        nop = mybir.InstNoOp(
                            name=f"{inst.name}-hw{j}",
                            sync_info=mybir.SyncInfo(on_wait=[w], on_update=[]),
                            bass_nofuse=True,
                            engine=inst.engine,
                        )
                        il.insert(i, nop)
                        i += 1
                    si.on_wait = waits[-1:]
                i += 1


def _classify_mask(mask2d):
    """Classify 128x128 blocks of the [S, S] bool mask (query s, key t).

    Returns (status[j][i], patterns) in scores-transposed coords:
    j = key(t) block, i = query(s) block. status: -1 skip, -2 full,
    >=0 index into patterns (additive bf16 [t, s] blocks, 0 or NEG).
    """
    status = [[-1] * NB for _ in range(NB)]
    patterns = []
    pat_idx = {}
    for j in range(NB):
        for i in range(NB):
            blk = mask2d[i * 128:(i + 1) * 128, j * 128:(j + 1) * 128]  # [s, t]
            if blk.all():
                status[j][i] = -2
            elif not blk.any():
                status[j][i] = -1
            else:
                add = np.where(blk.T, 0.0, NEG).astype(BF)  # [t, s]
                key = add.tobytes()
                if key not in pat_idx:
                    pat_idx[key] = len(patterns)
                    patterns.append(add)
                status[j][i] = pat_idx[key]
    return status, patterns


def _plan_chunks(status, patterns):
    """Per (c, j): suffix run of non-skip query blocks within chunk c.

    Returns plan[c][j] = (w, wide_pid or None) where w = run width and
    wide_pid indexes `wides`, full-region [128, 512] additive bf16 masks
    (left-justified to width w). Also first_j[c]. Asserts the
    suffix-nested structure the kernel relies on.
    """
    nch = S // 512
    plan = [[None] * NB for _ in range(nch)]
    first_j = [None] * nch
    wides = []
    wkeys = {}
    for c in range(nch):
        i_lo, i_hi = 4 * c, 4 * c + 4
        prev_w = None
        for j in range(NB):
            sts = [status[j][i] for i in range(i_lo, i_hi)]
            nz = [k for k, s in enumerate(sts) if s != -1]
            if not nz:
                plan[c][j] = (0, None)
                continue
            # must be a contiguous suffix of the chunk
            if nz != list(range(nz[0], 4)):
                raise NotImplementedError("mask block structure not suffix-contiguous")
            w = 128 * len(nz)
            if prev_w is not None and w > prev_w:
                raise NotImplementedError("mask runs not nested over key blocks")
            prev_w = w
            wide_pid = None
            if any(sts[k] >= 0 for k in nz):
                wide = np.zeros((128, 512), dtype=BF)
                for k in nz:
                    if sts[k] >= 0:
                        off = (k - nz[0]) * 128
                        wide[:, off:off + 128] = patterns[sts[k]]
                key = wide.tobytes()
                if key not in wkeys:
                    wkeys[key] = len(wides)
                    wides.append(wide)
                wide_pid = wkeys[key]
            plan[c][j] = (w, wide_pid)
            if first_j[c] is None:
                first_j[c] = j
    return plan, first_j, wides


def _build(plan, first_j, nwide, has_bk, has_bv, has_bo):
    nc = bass.Bass()
    qt = nc.dram_tensor("qt", [D, N], BF16, kind="ExternalInput")
    kt = nc.dram_tensor("kt", [D, N], BF16, kind="ExternalInput")
    vt = nc.dram_tensor("vt", [D, N], BF16, kind="ExternalInput")
    wkt = nc.dram_tensor("wkt", [D, D], BF16, kind="ExternalInput")
    wvt = nc.dram_tensor("wvt", [D, D], BF16, kind="ExternalInput")
    wot = nc.dram_tensor("wot", [D, D], BF16, kind="ExternalInput")
    bk32 = nc.dram_tensor("bk32", [128, 4], F32, kind="ExternalInput")
    bvb = nc.dram_tensor("bvb", [1, D], BF16, kind="ExternalInput")
    bob = nc.dram_tensor("bob", [1, D], BF16, kind="ExternalInput")
    ident = nc.dram_tensor("ident", [128, 128], BF16, kind="ExternalInput")
    mixadd = nc.dram_tensor("mixadd", [max(nwide, 1), 128, 512], BF16,
                            kind="ExternalInput")
    out = nc.dram_tensor("out", [N, D], F32, kind="ExternalOutput")

    nch = S // 512

    with tile.TileContext(nc) as tc:
        with ExitStack() as ctx:
            sing = ctx.enter_context(tc.tile_pool(name="sing", bufs=1))
            expp = ctx.enter_context(tc.tile_pool(name="expp", bufs=3))
            rcp = ctx.enter_context(tc.tile_pool(name="rcp", bufs=2))
            outp = ctx.enter_context(tc.tile_pool(name="outp", bufs=2))
            stp = ctx.enter_context(tc.tile_pool(name="stp", bufs=2, space="PSUM"))
            avp = ctx.enter_context(tc.tile_pool(name="avp", bufs=2, space="PSUM"))
            shp = ctx.enter_context(tc.tile_pool(name="shp", bufs=2, space="PSUM"))

            # ---- constant / input loads ----
            kt_sb = sing.tile([128, 4, N], BF16)
            nc.sync.dma_start(out=kt_sb, in_=kt.rearrange("(a p) n -> p a n", p=128))
            qt_sb = sing.tile([128, 4, N], BF16)
            nc.sync.dma_start(out=qt_sb, in_=qt.rearrange("(a p) n -> p a n", p=128))
            vt_sb = sing.tile([128, 4, N], BF16)
            nc.sync.dma_start(out=vt_sb, in_=vt.rearrange("(a p) n -> p a n", p=128))
            wkt_sb = sing.tile([128, 4, D], BF16)
            nc.sync.dma_start(out=wkt_sb, in_=wkt.rearrange("(a p) o -> p a o", p=128))
            wvt_sb = sing.tile([128, 4, D], BF16)
            nc.sync.dma_start(out=wvt_sb, in_=wvt.rearrange("(a p) o -> p a o", p=128))
            wot_sb = sing.tile([128, 4, D], BF16)
            nc.sync.dma_start(out=wot_sb, in_=wot.rearrange("(a p) o -> p a o", p=128))
            ident_sb = sing.tile([128, 128], BF16)
            nc.sync.dma_start(out=ident_sb, in_=ident[:, :])
            mix_sb = sing.tile([128, max(nwide, 1), 512], BF16)
            nc.sync.dma_start(out=mix_sb, in_=mixadd.rearrange("m t s -> t m s"))
            bk_sb = None
            if has_bk:
                bk_sb = sing.tile([128, 4], F32)
                nc.sync.dma_start(out=bk_sb, in_=bk32[:, :])
            bvb_sb = bob_sb = ones_k1 = None
            if has_bv or has_bo:
                ones_k1 = sing.tile([1, 128], BF16)
                nc.vector.memset(ones_k1, 1.0)
            if has_bv:
                bvb_sb = sing.tile([1, D], BF16)
                nc.sync.dma_start(out=bvb_sb, in_=bvb[:, :])
            if has_bo:
                bob_sb = sing.tile([1, D], BF16)
                nc.sync.dma_start(out=bob_sb, in_=bob[:, :])
            ones1 = sing.tile([1, 64], BF16)
            nc.vector.memset(ones1, 1.0)

            kp_sb = sing.tile([128, 4, N], BF16)
            qp_sb = sing.tile([128, 4, N], BF16)
            vp_sb = sing.tile([128, N // 128, 520], BF16)
            ct_sb = sing.tile([128, 4, N], BF16)

            # ---- k/q projections (feature-major [o, n]) ----
            for ob in range(4):
                for ch in range(N // 512):
                    csl = slice(ch * 512, ch * 512 + 512)
                    psK = shp.tile([128, 512], F32, tag="sh")
                    psQ = shp.tile([128, 512], F32, tag="sh")
                    for db in range(4):
                        nc.tensor.matmul(
                            psK, wkt_sb[:, db, ob * 128:(ob + 1) * 128],
                            kt_sb[:, db, csl], start=(db == 0), stop=(db == 3))
                    for db in range(4):
                        nc.tensor.matmul(
                            psQ, wkt_sb[:, db, ob * 128:(ob + 1) * 128],
                            qt_sb[:, db, csl], start=(db == 0), stop=(db == 3))
                    if has_bk:
                        nc.scalar.add(kp_sb[:, ob, csl], psK, bk_sb[:, ob:ob + 1])
                        nc.vector.tensor_scalar_add(
                            qp_sb[:, ob, csl], psQ, bk_sb[:, ob:ob + 1])
                    else:
                        nc.scalar.copy(kp_sb[:, ob, csl], psK)
                        nc.vector.tensor_copy(out=qp_sb[:, ob, csl], in_=psQ)

            # ---- v projection (token-major [n, o], stride-65 + ones cols) ----
            for nt in range(N // 128):
                nc.vector.memset(
                    vp_sb[:, nt, :].rearrange("p (h u) -> p h u", u=65)[:, :, 64:65],
                    1.0)
            for nt in range(N // 128):
                psV = shp.tile([128, 512], F32, tag="sh")
                for db in range(4):
                    nc.tensor.matmul(
                        psV, vt_sb[:, db, nt * 128:(nt + 1) * 128],
                        wvt_sb[:, db, :], start=(db == 0),
                        stop=(db == 3 and not has_bv))
                if has_bv:
                    nc.tensor.matmul(psV, ones_k1, bvb_sb[0:1, :],
                                     start=False, stop=True)
                dst = vp_sb[:, nt, :].rearrange("p (h u) -> p h u", u=65)[:, :, 0:64]
                src = psV.rearrange("p (h u) -> p h u", u=64)
                if nt % 2 == 0:
                    nc.vector.tensor_copy(out=dst, in_=src)
                else:
                    nc.scalar.copy(dst, src)

            # ---- attention + output projection, per local batch ----
            for b in range(BL):
                for hp in range(HP):
                    h0, h1 = 2 * hp, 2 * hp + 1
                    for c in range(nch):
                        fj = first_j[c]
                        if fj is None:
                            continue
                        av0 = avp.tile([65, 512], F32, tag="av")
                        av1 = avp.tile([65, 512], F32, tag="av")
                        js = [j for j in range(NB) if plan[c][j][0] > 0]
                        # software-pipelined: scores(j+1) emitted before AV(j)
                        st_t = {}

                        def scores(j):
                            w, wide_pid = plan[c][j]
                            st = stp.tile([128, 1024], F32, tag="st")
                            st_t[j] = st
                            tsl = slice(b * S + j * 128, b * S + j * 128 + 128)
                            ssl = slice(b * S + c * 512 + 512 - w,
                                        b * S + c * 512 + 512)
                            has_m = wide_pid is not None
                            nc.tensor.matmul(st[:, 512 - w:512],
                                             kp_sb[0:64, hp, tsl],
                                             qp_sb[0:64, hp, ssl],
                                             start=True, stop=not has_m)
                            nc.tensor.matmul(st[:, 512:512 + w],
                                             kp_sb[64:128, hp, tsl],
                                             qp_sb[64:128, hp, ssl],
                                             start=True, stop=not has_m)
                            if has_m:
                                nc.tensor.matmul(
                                    st[:, 512 - w:512],
                                    ident_sb, mix_sb[:, wide_pid, 0:w],
                                    start=False, stop=True)
                                nc.tensor.matmul(
                                    st[:, 512:512 + w],
                                    ident_sb, mix_sb[:, wide_pid, 0:w],
                                    start=False, stop=True)

                        scores(js[0])
                        for jx, j in enumerate(js):
                            w, _ = plan[c][j]
                            st = st_t.pop(j)
                            ex = expp.tile([128, 1024], BF16)
                            nc.scalar.activation(
                                ex[:, 0:2 * w], st[:, 512 - w:512 + w],
                                mybir.ActivationFunctionType.Exp, scale=0.125)
                            if jx + 1 < len(js):
                                scores(js[jx + 1])
                            vrow = b * NB + j
                            nc.tensor.matmul(
                                av0[:, 512 - w:512],
                                vp_sb[:, vrow, 65 * h0:65 * h0 + 65],
                                ex[:, 0:w], start=(j == fj),
                                stop=(jx == len(js) - 1))
                            nc.tensor.matmul(
                                av1[:, 512 - w:512],
                                vp_sb[:, vrow, 65 * h1:65 * h1 + 65],
                                ex[:, w:2 * w], start=(j == fj),
                                stop=(jx == len(js) - 1))

                        rc = rcp.tile([1, 1024], BF16)
                        with nc.allow_low_precision(reason="softmax recip bf16"):
                            nc.vector.reciprocal(out=rc[0:1, 0:512],
                                                 in_=av0[64:65, :])
                            nc.vector.reciprocal(out=rc[0:1, 512:1024],
                                                 in_=av1[64:65, :])
                        bc = shp.tile([128, 512], F32, tag="sh")
                        nc.tensor.matmul(bc[0:64, :], ones1, rc[0:1, 0:512],
                                         start=True, stop=True)
                        nc.tensor.matmul(bc[64:128, :], ones1, rc[0:1, 512:1024],
                                         start=True, stop=True, tile_position=(0, 64))
                        bcs = rcp.tile([128, 512], BF16, tag="bcs")
                        if (b + hp + c) % 2 == 0:
                            nc.scalar.copy(bcs, bc)
                        else:
                            nc.vector.tensor_copy(out=bcs, in_=bc)
                        osl = slice(b * S + c * 512, b * S + c * 512 + 512)
                        nc.vector.tensor_mul(ct_sb[0:64, hp, osl],
                                             av0[0:64, :], bcs[0:64, :])
                        nc.vector.tensor_mul(ct_sb[64:128, hp, osl],
                                             av1[0:64, :], bcs[64:128, :])

                # ---- output projection for batch b ----
                for nt in range(NB):
                    gnt = b * NB + nt
                    psO = shp.tile([128, 512], F32, tag="sh")
                    for hp in range(4):
                        nc.tensor.matmul(
                            psO, ct_sb[:, hp, gnt * 128:(gnt + 1) * 128],
                            wot_sb[:, hp, :], start=(hp == 0),
                            stop=(hp == 3 and not has_bo))
                    if has_bo:
                        nc.tensor.matmul(psO, ones_k1, bob_sb[0:1, :],
                                         start=False, stop=True)
                    ot = outp.tile([128, 512], F32)
                    if nt % 2 == 0:
                        nc.vector.tensor_copy(out=ot, in_=psO)
                    else:
                        nc.scalar.copy(ot, psO)
                    nc.sync.dma_start(out=out[gnt * 128:(gnt + 1) * 128, :], in_=ot)

    return nc


_prog_cache = {}


def kernel(q, k, v, mask, zero_pad, Wk, bk, Wv, bv, Wo, bo):
    global LAST_SIM_NS, LAST_EXEC_NS
    q = np.asarray(q, dtype=np.float32)
    k = np.asarray(k, dtype=np.float32)
    v = np.asarray(v, dtype=np.float32)
    Wk = np.asarray(Wk, dtype=np.float32)
    Wv = np.asarray(Wv, dtype=np.float32)
    Wo = np.asarray(Wo, dtype=np.float32)
    bk = np.asarray(bk, dtype=np.float32).reshape(D)
    bv = np.asarray(bv, dtype=np.float32).reshape(D)
    bo = np.asarray(bo, dtype=np.float32).reshape(D)
    mask2d = np.asarray(mask).reshape(S, S).astype(bool)
    zp = int(np.asarray(zero_pad))

    status, patterns = _classify_mask(mask2d)
    plan, first_j, wides = _plan_chunks(status, patterns)
    nwide = len(wides)
    has_bk = bool(np.any(bk))
    has_bv = bool(np.any(bv))
    has_bo = bool(np.any(bo))

    sig = (tuple(tuple(r) for r in status), nwide, has_bk, has_bv, has_bo)
    if sig not in _prog_cache:
        nc_new = _build(plan, first_j, nwide, has_bk, has_bv, has_bo)
        legalize_waits(nc_new)   # hardware-only pass (sim runs pre-legalized)
        _prog_cache[sig] = nc_new
    nc = _prog_cache[sig]

    wkt = np.ascontiguousarray(Wk.T).astype(BF)
    wvt = np.ascontiguousarray(Wv.T).astype(BF)
    wot = np.ascontiguousarray(Wo.T).astype(BF)
    bk32 = np.ascontiguousarray(bk.reshape(4, 128).T).astype(np.float32)
    bvb = bv.reshape(1, D).astype(BF)
    bob = bo.reshape(1, D).astype(BF)
    ident = np.eye(128, dtype=BF)
    mixadd = (np.stack(wides) if wides
              else np.zeros((1, 128, 512), np.float32)).astype(BF)

    common = dict(wkt=wkt, wvt=wvt, wot=wot, bk32=bk32, bvb=bvb, bob=bob,
                  ident=ident, mixadd=mixadd)
    in_maps = []
    for ci in range(NCORES):
        sl = slice(ci * BL, (ci + 1) * BL)
        in_maps.append(dict(
            qt=np.ascontiguousarray(q[sl].reshape(N, D).T).astype(BF),
            kt=np.ascontiguousarray(k[sl].reshape(N, D).T).astype(BF),
            vt=np.ascontiguousarray(v[sl].reshape(N, D).T).astype(BF),
            **common))

    if os.environ.get("BASS_KERNEL_SIM_TIME"):
        from concourse.timeline_sim import TimelineSim
        LAST_SIM_NS = TimelineSim(nc).simulate()

    res = run_bass_kernel_spmd(nc, in_maps, list(range(NCORES)))
    LAST_EXEC_NS = res.exec_time_ns

    outs = [res.results[ci]["out"].reshape(BL, S, D) for ci in range(NCORES)]
    full = np.concatenate(outs, axis=0)
    if zp:
        full[:, 0, :] = bo
    return full


# revision 24
# speedup vs baseline: 1.1941x; 1.1941x over previous
"""Trainium2 Bass kernel for nn_MultiHeadAttention_47579647705431.

Multi-head attention (8 heads, dim 512, seq 1024, batch 16) with:
  - shared key/query linear (key_query_same=True: q and k both use Wk/bk)
  - causal (or arbitrary block-structured) mask
  - SimpleKT zero_pad: attention row 0 zeroed => out[:, 0, :] = bo

Sharding: data-parallel over batch across 8 NeuronCores (2 batches/core).
Per-core device pipeline (all matmuls bf16, fp32 PSUM accumulation):
  host: cast+transpose q/k/v slices and weights to bf16 feature-major
  1. kp/qp = Wk.T-stationary projections -> feature-major [o, n] bf16
  2. vp   = token-major projection [n, o] bf16 with interleaved ones
           columns (stride-65 layout) for the denominator ones-row trick
  3. per (batch, head-pair, s-chunk): scores^T [t, s] via K=64 row-packed
     matmuls, additive -1e9 mask via identity-matmul accumulation on mixed
     blocks, exp on ACT (scale=1/8 folded), AV via [t, 65]-stationary
     matmuls accumulating [65, 512] PSUM whose row 64 is the softmax
     denominator; reciprocal + K=1 ones broadcast matmuls + DVE multiply
     normalize into concat^T bf16
  4. final projection (concat^T-stationary) -> [n, o] fp32 -> DRAM

The walrus build here supports ONE sync wait per instruction; Tile emits
more. legalize_waits() hoists extra waits onto same-engine NoOps.
"""

import os
from contextlib import ExitStack

import numpy as np
import ml_dtypes

import concourse.bass as bass
import concourse.mybir as mybir
import concourse.tile as tile
from concourse.bass_utils import run_bass_kernel_spmd

F32 = mybir.dt.float32
BF16 = mybir.dt.bfloat16
BF = ml_dtypes.bfloat16

B, S, D, H, DH = 16, 1024, 512, 8, 64
NCORES = 8
BL = B // NCORES          # batches per core
N = BL * S                # tokens per core
NB = S // 128             # 128-blocks per sequence (8)
HP = H // 2               # head pairs (= o-blocks of 128)
NEG = -1.0e9

LAST_SIM_NS = None
LAST_EXEC_NS = None


def legalize_waits(nc):
    """Split multi-wait instructions: keep one wait, hoist the rest onto
    preceding same-engine NoOps (this walrus encodes 1 wait/instruction)."""
    for f in nc.m.functions:
        for blk in f.blocks:
            il = blk.instructions
            i = 0
            while i < len(il):
                inst = il[i]
                si = inst.sync_info
                if si is not None and si.on_wait and len(si.on_wait) > 1:
                    waits = list(si.on_wait)
                    for j, w in enumerate(waits[:-1]):
                        nop = mybir.InstNoOp(
                            name=f"{inst.name}-hw{j}",
                            sync_info=mybir.SyncInfo(on_wait=[w], on_update=[]),
                            bass_nofuse=True,
                            engine=inst.engine,
                        )
                        il.insert(i, nop)
                        i += 1
                    si.on_wait = waits[-1:]
                i += 1


def _classify_mask(mask2d):
    """Classify 128x128 blocks of the [S, S] bool mask (query s, key t).

    Returns (status[j][i], patterns) in scores-transposed coords:
    j = key(t) block, i = query(s) block. status: -1 skip, -2 full,
    >=0 index into patterns (additive bf16 [t, s] blocks, 0 or NEG).
    """
    status = [[-1] * NB for _ in range(NB)]
    patterns = []
    pat_idx = {}
    for j in range(NB):
        for i in range(NB):
            blk = mask2d[i * 128:(i + 1) * 128, j * 128:(j + 1) * 128]  # [s, t]
            if blk.all():
                status[j][i] = -2
            elif not blk.any():
                status[j][i] = -1
            else:
                add = np.where(blk.T, 0.0, NEG).astype(BF)  # [t, s]
                key = add.tobytes()
                if key not in pat_idx:
                    pat_idx[key] = len(patterns)
                    patterns.append(add)
                status[j][i] = pat_idx[key]
    return status, patterns


def _plan_chunks(status, patterns):
    """Per (c, j): suffix run of non-skip query blocks within chunk c.

    Returns plan[c][j] = (w, mixes) where w = run width and mixes =
    [(col_offset_in_region, pattern_id), ...] for mixed blocks. Also
    first_j[c]. Asserts the suffix-nested structure the kernel relies on.
    """
    nch = S // 512
    plan = [[None] * NB for _ in range(nch)]
    first_j = [None] * nch
    for c in range(nch):
        i_lo, i_hi = 4 * c, 4 * c + 4
        prev_w = None
        for j in range(NB):
            sts = [status[j][i] for i in range(i_lo, i_hi)]
            nz = [k for k, s in enumerate(sts) if s != -1]
            if not nz:
                plan[c][j] = (0, [])
                continue
            # must be a contiguous suffix of the chunk
            if nz != list(range(nz[0], 4)):
                raise NotImplementedError("mask block structure not suffix-contiguous")
            w = 128 * len(nz)
            if prev_w is not None and w > prev_w:
                raise NotImplementedError("mask runs not nested over key blocks")
            prev_w = w
            mixes = [((k - nz[0]) * 128, sts[k]) for k in nz if sts[k] >= 0]
            plan[c][j] = (w, mixes)
            if first_j[c] is None:
                first_j[c] = j
    return plan, first_j


def _build(plan, first_j, nmix, has_bk, has_bv, has_bo):
    nc = bass.Bass()
    qt = nc.dram_tensor("qt", [128, 4, N], BF16, kind="ExternalInput")
    kt = nc.dram_tensor("kt", [128, 4, N], BF16, kind="ExternalInput")
    vt = nc.dram_tensor("vt", [128, 4, N], BF16, kind="ExternalInput")
    wkt = nc.dram_tensor("wkt", [128, 4, D], BF16, kind="ExternalInput")
    wvt = nc.dram_tensor("wvt", [128, 4, D], BF16, kind="ExternalInput")
    wot = nc.dram_tensor("wot", [128, 4, D], BF16, kind="ExternalInput")
    bk32 = nc.dram_tensor("bk32", [128, 4], F32, kind="ExternalInput")
    bvb = nc.dram_tensor("bvb", [1, D], BF16, kind="ExternalInput")
    bob = nc.dram_tensor("bob", [1, D], BF16, kind="ExternalInput")
    ident = nc.dram_tensor("ident", [128, 128], BF16, kind="ExternalInput")
    mixadd = nc.dram_tensor("mixadd", [max(nmix, 1), 128, 128], BF16,
                            kind="ExternalInput")
    out = nc.dram_tensor("out", [N, D], F32, kind="ExternalOutput")

    nch = S // 512

    with tile.TileContext(nc) as tc:
        with ExitStack() as ctx:
            sing = ctx.enter_context(tc.tile_pool(name="sing", bufs=1))
            expp = ctx.enter_context(tc.tile_pool(name="expp", bufs=5))
            rcp = ctx.enter_context(tc.tile_pool(name="rcp", bufs=3))
            outp = ctx.enter_context(tc.tile_pool(name="outp", bufs=3))
            stp = ctx.enter_context(tc.tile_pool(name="stp", bufs=2, space="PSUM"))
            avp = ctx.enter_context(tc.tile_pool(name="avp", bufs=2, space="PSUM"))
            shp = ctx.enter_context(tc.tile_pool(name="shp", bufs=2, space="PSUM"))

            # ---- input loads: critical-path first (kt/qt ch0 gate proj(0)),
            # cold constants (wot/ident/mixadd) last on the shared DMA device
            wkt_sb = sing.tile([128, 4, D], BF16)
            nc.sync.dma_start(out=wkt_sb, in_=wkt[:, :, :])
            wvt_sb = sing.tile([128, 4, D], BF16)
            nc.gpsimd.dma_start(out=wvt_sb, in_=wvt[:, :, :])
            kt_c, qt_c, vt_c = [], [], []
            for ch in range(4):
                csl = slice(ch * 512, ch * 512 + 512)
                t = sing.tile([128, 4, 512], BF16, tag=f"ktc{ch}")
                nc.sync.dma_start(out=t, in_=kt[:, :, csl])
                kt_c.append(t)
                t = sing.tile([128, 4, 512], BF16, tag=f"qtc{ch}")
                nc.scalar.dma_start(out=t, in_=qt[:, :, csl])
                qt_c.append(t)
                t = sing.tile([128, 4, 512], BF16, tag=f"vtc{ch}")
                nc.gpsimd.dma_start(out=t, in_=vt[:, :, csl])
                vt_c.append(t)
            ident_sb = sing.tile([128, 128], BF16)
            nc.sync.dma_start(out=ident_sb, in_=ident[:, :])
            mix_sb = sing.tile([128, max(nmix, 1), 128], BF16)
            nc.sync.dma_start(out=mix_sb, in_=mixadd.rearrange("m t s -> t m s"))
            wot_sb = sing.tile([128, 4, D], BF16)
            nc.scalar.dma_start(out=wot_sb, in_=wot[:, :, :])
            bk_sb = None
            if has_bk:
                bk_sb = sing.tile([128, 4], F32)
                nc.sync.dma_start(out=bk_sb, in_=bk32[:, :])
            bvb_sb = bob_sb = ones_k1 = None
            if has_bv or has_bo:
                ones_k1 = sing.tile([1, 128], BF16)
                nc.vector.memset(ones_k1, 1.0)
            if has_bv:
                bvb_sb = sing.tile([1, D], BF16)
                nc.sync.dma_start(out=bvb_sb, in_=bvb[:, :])
            if has_bo:
                bob_sb = sing.tile([1, D], BF16)
                nc.sync.dma_start(out=bob_sb, in_=bob[:, :])
            ones1 = sing.tile([1, 64], BF16)
            nc.vector.memset(ones1, 1.0)

            kp_sb = sing.tile([128, 4, N], BF16)
            qp_sb = sing.tile([128, 4, N], BF16)
            vp_sb = sing.tile([128, N // 128, 520], BF16)
            ct_sb = sing.tile([128, 4, N], BF16)

            # ---- phase functions (emitted interleaved for engine overlap) ----
            fillers = []

            def kq_group(ob, ch):
                    csl = slice(ch * 512, ch * 512 + 512)
                    psK = shp.tile([128, 512], F32, tag="sh")
                    psQ = shp.tile([128, 512], F32, tag="sh")
                    for db in range(4):
                        nc.tensor.matmul(
                            psK, wkt_sb[:, db, ob * 128:(ob + 1) * 128],
                            kt_c[ch][:, db, :], start=(db == 0), stop=(db == 3))
                    for db in range(4):
                        nc.tensor.matmul(
                            psQ, wkt_sb[:, db, ob * 128:(ob + 1) * 128],
                            qt_c[ch][:, db, :], start=(db == 0), stop=(db == 3))
                    if has_bk:
                        nc.scalar.add(kp_sb[:, ob, csl], psK, bk_sb[:, ob:ob + 1])
                        nc.vector.tensor_scalar_add(
                            qp_sb[:, ob, csl], psQ, bk_sb[:, ob:ob + 1])
                    elif (ob + ch) % 2 == 0:
                        nc.scalar.copy(kp_sb[:, ob, csl], psK)
                        nc.vector.tensor_copy(out=qp_sb[:, ob, csl], in_=psQ)
                    else:
                        nc.vector.tensor_copy(out=kp_sb[:, ob, csl], in_=psK)
                        nc.scalar.copy(qp_sb[:, ob, csl], psQ)

            def kq_proj(ob):
                for ch in range(N // 512):
                    kq_group(ob, ch)

            def v_proj(nt):
                psV = shp.tile([128, 512], F32, tag="sh")
                for db in range(4):
                    nc.tensor.matmul(
                        psV, vt_c[nt // 4][:, db, (nt % 4) * 128:(nt % 4) * 128 + 128],
                        wvt_sb[:, db, :], start=(db == 0),
                        stop=(db == 3 and not has_bv))
                if has_bv:
                    nc.tensor.matmul(psV, ones_k1, bvb_sb[0:1, :],
                                     start=False, stop=True)
                dst = vp_sb[:, nt, :].rearrange("p (h u) -> p h u", u=65)[:, :, 0:64]
                src = psV.rearrange("p (h u) -> p h u", u=64)
                if nt % 2 == 0:
                    nc.vector.tensor_copy(out=dst, in_=src)
                else:
                    nc.scalar.copy(dst, src)

            st_store = {}

            def attention_c(b, hp, c, next_start=None):
                h0, h1 = 2 * hp, 2 * hp + 1
                if True:
                    fj = first_j[c]
                    if fj is None:
                        return
                    av0 = avp.tile([65, 512], F32, tag="av")
                    av1 = avp.tile([65, 512], F32, tag="av")
                    js = [j for j in range(NB) if plan[c][j][0] > 0]
                    st_t = st_store.setdefault((b, hp, c), {})

                    def scores(j):
                        w, mixes = plan[c][j]
                        st = stp.tile([128, 1024], F32, tag="st")
                        st_t[j] = st
                        tsl = slice(b * S + j * 128, b * S + j * 128 + 128)
                        ssl = slice(b * S + c * 512 + 512 - w,
                                    b * S + c * 512 + 512)
                        nc.tensor.matmul(st[:, 512 - w:512],
                                         kp_sb[0:64, hp, tsl],
                                         qp_sb[0:64, hp, ssl],
                                         start=True, stop=not mixes)
                        nc.tensor.matmul(st[:, 512:512 + w],
                                         kp_sb[64:128, hp, tsl],
                                         qp_sb[64:128, hp, ssl],
                                         start=True, stop=not mixes)
                        for mi, (off, pid) in enumerate(mixes):
                            last = mi == len(mixes) - 1
                            o0 = 512 - w + off
                            nc.tensor.matmul(st[:, o0:o0 + 128], ident_sb,
                                             mix_sb[:, pid, :],
                                             start=False, stop=last)
                            nc.tensor.matmul(st[:, 512 + off:512 + off + 128],
                                             ident_sb, mix_sb[:, pid, :],
                                             start=False, stop=last)

                    if js[0] not in st_t:
                        scores(js[0])
                    for jx, j in enumerate(js):
                        w, _ = plan[c][j]
                        st = st_t.pop(j)
                        ex = expp.tile([128, 1024], BF16)
                        nc.scalar.activation(
                            ex[:, 0:2 * w], st[:, 512 - w:512 + w],
                            mybir.ActivationFunctionType.Exp, scale=0.125)
                        if jx + 1 < len(js):
                            scores(js[jx + 1])
                        elif next_start is not None:
                            next_start()
                        vrow = b * NB + j
                        nc.tensor.matmul(
                            av0[:, 512 - w:512],
                            vp_sb[:, vrow, 65 * h0:65 * h0 + 65],
                            ex[:, 0:w], start=(j == fj),
                            stop=(jx == len(js) - 1))
                        nc.tensor.matmul(
                            av1[:, 512 - w:512],
                            vp_sb[:, vrow, 65 * h1:65 * h1 + 65],
                            ex[:, w:2 * w], start=(j == fj),
                            stop=(jx == len(js) - 1))
                        if fillers:
                            fillers.pop(0)()

                    rc = rcp.tile([1, 1024], BF16)
                    with nc.allow_low_precision(reason="softmax recip bf16"):
                        nc.vector.reciprocal(out=rc[0:1, 0:512],
                                             in_=av0[64:65, :])
                        nc.vector.reciprocal(out=rc[0:1, 512:1024],
                                             in_=av1[64:65, :])

                    def epilogue(rc=rc, av0=av0, av1=av1, b=b, hp=hp, c=c):
                        # deferred into the next block's filler slots so the
                        # PE bc matmuls don't stall on the reciprocal latency
                        bc = shp.tile([128, 512], F32, tag="sh")
                        nc.tensor.matmul(bc[0:64, :], ones1, rc[0:1, 0:512],
                                         start=True, stop=True)
                        nc.tensor.matmul(bc[64:128, :], ones1,
                                         rc[0:1, 512:1024],
                                         start=True, stop=True,
                                         tile_position=(0, 64))
                        bcs = rcp.tile([128, 512], BF16, tag="bcs")
                        nc.vector.tensor_copy(out=bcs, in_=bc)
                        osl = slice(b * S + c * 512, b * S + c * 512 + 512)
                        nc.vector.tensor_mul(ct_sb[0:64, hp, osl],
                                             av0[0:64, :], bcs[0:64, :])
                        nc.vector.tensor_mul(ct_sb[64:128, hp, osl],
                                             av1[0:64, :], bcs[64:128, :])

                    fillers.insert(0, epilogue)

            def flush_fillers():
                while fillers:
                    fillers.pop(0)()

            def final_half(b, c):
                for nt in range(4 * c, 4 * c + 4):
                    gnt = b * NB + nt
                    psO = shp.tile([128, 512], F32, tag="sh")
                    for hp in range(4):
                        nc.tensor.matmul(
                            psO, ct_sb[:, hp, gnt * 128:(gnt + 1) * 128],
                            wot_sb[:, hp, :], start=(hp == 0),
                            stop=(hp == 3 and not has_bo))
                    if has_bo:
                        nc.tensor.matmul(psO, ones_k1, bob_sb[0:1, :],
                                         start=False, stop=True)
                    ot = outp.tile([128, 512], F32)
                    if nt % 2 == 0:
                        nc.vector.tensor_copy(out=ot, in_=psO)
                        nc.scalar.dma_start(
                            out=out[gnt * 128:(gnt + 1) * 128, :], in_=ot)
                    else:
                        nc.scalar.copy(ot, psO)
                        nc.sync.dma_start(
                            out=out[gnt * 128:(gnt + 1) * 128, :], in_=ot)

            # ones columns of vp
            for nt in range(N // 128):
                nc.vector.memset(
                    vp_sb[:, nt, :].rearrange("p (h u) -> p h u", u=65)[:, :, 64:65],
                    1.0)

            # interleaved emission: ACT exp work starts as early as possible;
            # attention split by s-chunk so final-projection halves overlap
            kq_proj(0)
            for nt in range(NB):
                v_proj(nt)
            import functools
            for nt in range(NB, N // 128):
                fillers.append(functools.partial(v_proj, nt))
            for ob in (1, 2, 3):
                for ch in range(N // 512):
                    fillers.append(functools.partial(kq_group, ob, ch))
            seq = [(0, 0, 1), (1, 0, 1), (0, 1, 1), (1, 1, 1), (0, 2, 1),
                   (1, 2, 1), (0, 3, 1), (1, 3, 1), (0, 0, 0), (1, 0, 0),
                   (0, 1, 0), (1, 1, 0), (0, 2, 0), (1, 2, 0), (0, 3, 0),
                   (1, 3, 0)]

            def make_next_start(nb_, nhp_, nc_):
                def _start():
                    # emit the next block's first scores inside this block so
                    # PE keeps ACT fed across the block boundary
                    h0_, h1_ = 2 * nhp_, 2 * nhp_ + 1
                    js_ = [j for j in range(NB) if plan[nc_][j][0] > 0]
                    if not js_:
                        return
                    j = js_[0]
                    w, mixes = plan[nc_][j]
                    st = stp.tile([128, 1024], F32, tag="st")
                    st_store.setdefault((nb_, nhp_, nc_), {})[j] = st
                    tsl = slice(nb_ * S + j * 128, nb_ * S + j * 128 + 128)
                    ssl = slice(nb_ * S + nc_ * 512 + 512 - w,
                                nb_ * S + nc_ * 512 + 512)
                    nc.tensor.matmul(st[:, 512 - w:512],
                                     kp_sb[0:64, nhp_, tsl],
                                     qp_sb[0:64, nhp_, ssl],
                                     start=True, stop=not mixes)
                    nc.tensor.matmul(st[:, 512:512 + w],
                                     kp_sb[64:128, nhp_, tsl],
                                     qp_sb[64:128, nhp_, ssl],
                                     start=True, stop=not mixes)
                    for mi, (off, pid) in enumerate(mixes):
                        last = mi == len(mixes) - 1
                        o0 = 512 - w + off
                        nc.tensor.matmul(st[:, o0:o0 + 128], ident_sb,
                                         mix_sb[:, pid, :],
                                         start=False, stop=last)
                        nc.tensor.matmul(st[:, 512 + off:512 + off + 128],
                                         ident_sb, mix_sb[:, pid, :],
                                         start=False, stop=last)
                return _start

            for bi, (b_, hp_, c_) in enumerate(seq):
                nxt = make_next_start(*seq[bi + 1]) if bi + 1 < len(seq) else None
                attention_c(b_, hp_, c_, next_start=nxt)
                if (b_, hp_, c_) == (0, 3, 1):
                    flush_fillers()
                    final_half(0, 1)
                elif (b_, hp_, c_) == (1, 3, 1):
                    flush_fillers()
                    final_half(1, 1)
                elif (b_, hp_, c_) == (0, 3, 0):
                    flush_fillers()
                    final_half(0, 0)
                elif (b_, hp_, c_) == (1, 3, 0):
                    flush_fillers()
                    final_half(1, 0)

    return nc


_prog_cache = {}


def kernel(q, k, v, mask, zero_pad, Wk, bk, Wv, bv, Wo, bo):
    global LAST_SIM_NS, LAST_EXEC_NS
    q = np.asarray(q, dtype=np.float32)
    k = np.asarray(k, dtype=np.float32)
    v = np.asarray(v, dtype=np.float32)
    Wk = np.asarray(Wk, dtype=np.float32)
    Wv = np.asarray(Wv, dtype=np.float32)
    Wo = np.asarray(Wo, dtype=np.float32)
    bk = np.asarray(bk, dtype=np.float32).reshape(D)
    bv = np.asarray(bv, dtype=np.float32).reshape(D)
    bo = np.asarray(bo, dtype=np.float32).reshape(D)
    mask2d = np.asarray(mask).reshape(S, S).astype(bool)
    zp = int(np.asarray(zero_pad))

    status, patterns = _classify_mask(mask2d)
    plan, first_j = _plan_chunks(status, patterns)
    nmix = len(patterns)
    has_bk = bool(np.any(bk))
    has_bv = bool(np.any(bv))
    has_bo = bool(np.any(bo))

    sig = (tuple(tuple(r) for r in status), nmix, has_bk, has_bv, has_bo)
    if sig not in _prog_cache:
        nc_new = _build(plan, first_j, nmix, has_bk, has_bv, has_bo)
        legalize_waits(nc_new)   # hardware-only pass (sim runs pre-legalized)
        _prog_cache[sig] = nc_new
    nc = _prog_cache[sig]

    def _sbuf_layout(wt):
        # [D, X] -> [128, 4, X]: row d = a*128+p  ->  [p, a, :]
        return np.ascontiguousarray(wt.reshape(4, 128, -1).transpose(1, 0, 2))

    wkt = _sbuf_layout(Wk.T.astype(BF))
    wvt = _sbuf_layout(Wv.T.astype(BF))
    wot = _sbuf_layout(Wo.T.astype(BF))
    bk32 = np.ascontiguousarray(bk.reshape(4, 128).T).astype(np.float32)
    bvb = bv.reshape(1, D).astype(BF)
    bob = bo.reshape(1, D).astype(BF)
    ident = np.eye(128, dtype=BF)
    mixadd = (np.stack(patterns) if patterns
              else np.zeros((1, 128, 128), np.float32)).astype(BF)

    common = dict(wkt=wkt, wvt=wvt, wot=wot, bk32=bk32, bvb=bvb, bob=bob,
                  ident=ident, mixadd=mixadd)
    in_maps = []
    for ci in range(NCORES):
        sl = slice(ci * BL, (ci + 1) * BL)
        in_maps.append(dict(
            qt=_sbuf_layout(q[sl].reshape(N, D).T.astype(BF)),
            kt=_sbuf_layout(k[sl].reshape(N, D).T.astype(BF)),
            vt=_sbuf_layout(v[sl].reshape(N, D).T.astype(BF)),
            **common))

    if os.environ.get("BASS_KERNEL_SIM_TIME"):
        from concourse.timeline_sim import TimelineSim
        LAST_SIM_NS = TimelineSim(nc).simulate()

    res = run_bass_kernel_spmd(nc, in_maps, list(range(NCORES)))
    LAST_EXEC_NS = res.exec_time_ns

    outs = [res.results[ci]["out"].reshape(BL, S, D) for ci in range(NCORES)]
    full = np.concatenate(outs, axis=0)
    if zp:
        full[:, 0, :] = bo
    return full
